# revision 1
# baseline (speedup 1.0000x reference)
"""Trainium2 Bass kernel for 16-head MultiHeadAttention (B=2, S=2048, D=1024).

Sharding: 8 cores = 2 (batch) x 4 (head groups of 4 heads).
Each core computes qkv projection for its 4 heads, attention, and a partial
out-projection (TP over heads); host sums the 4 partials per batch element.

Self-contained: hardcodes shapes; only dependency is the in-container
concourse/bass stack at /opt/trn_rl_repo.
"""

import os
import sys
from dataclasses import dataclass

for _p in ("/opt/trn_rl_repo",):
    if _p not in sys.path:
        sys.path.insert(0, _p)

import numpy as np

import concourse.bass as bass  # noqa: E402
import concourse.bacc as bacc  # noqa: E402
import concourse.tile as tile  # noqa: E402
from concourse import mybir  # noqa: E402
from concourse.bass_utils import run_bass_kernel_spmd  # noqa: E402

F32 = mybir.dt.float32
BF16 = mybir.dt.bfloat16
AF = mybir.ActivationFunctionType

# Set False if gpsimd partition_broadcast is unavailable; falls back to a
# PE ones-matmul broadcast.
USE_GPSIMD_BCAST = True


@dataclass(frozen=True)
class Cfg:
    S: int = 2048      # sequence length
    DIN: int = 1024    # model dim
    HPC: int = 4       # heads per core
    DK: int = 64       # head dim
    N_CORES: int = 8

    @property
    def DQK(self):
        return self.HPC * self.DK  # 256: per-core Q (and K, V) output dim

    @property
    def KC(self):
        return self.DIN // 128     # contraction chunks for projections

    @property
    def SB(self):
        return self.S // 128       # 128-row blocks of the sequence

    @property
    def SQC(self):
        return min(1024, self.S)   # query-column chunk for attention

    @property
    def MMN(self):
        return min(512, self.S)    # matmul moving free-dim chunk


FULL = Cfg()


def build_nc(cfg: Cfg = FULL):
    S, DIN, HPC, DK = cfg.S, cfg.DIN, cfg.HPC, cfg.DK
    DQK, KC, SB, SQC, MMN = cfg.DQK, cfg.KC, cfg.SB, cfg.SQC, cfg.MMN
    NQ = S // SQC
    N2 = SQC // MMN          # moving chunks per SQC
    NSC = S // MMN           # s chunks for projections
    NDC = max(1, DIN // 512)  # out-proj dout chunks
    ODC = DIN // NDC
    VC = DQK // 128          # head-dim chunks for out-proj contraction (2)
    SCALE_INV = 1.0 / float(np.sqrt(DK))

    nc = bacc.Bacc("TRN2", target_bir_lowering=False, debug=False,
                   num_devices=cfg.N_CORES)

    x_d = nc.dram_tensor("x", [S, DIN], F32, kind="ExternalInput")
    wq_d = nc.dram_tensor("w_q", [DQK, DIN], F32, kind="ExternalInput")
    wk_d = nc.dram_tensor("w_k", [DQK, DIN], F32, kind="ExternalInput")
    wv_d = nc.dram_tensor("w_v", [DQK, DIN], F32, kind="ExternalInput")
    bq_d = nc.dram_tensor("b_q", [DQK, 1], F32, kind="ExternalInput")
    bk_d = nc.dram_tensor("b_k", [DQK, 1], F32, kind="ExternalInput")
    bv_d = nc.dram_tensor("b_v", [1, DQK], F32, kind="ExternalInput")
    wo_d = nc.dram_tensor("w_o", [DIN, DQK], F32, kind="ExternalInput")
    bo_d = nc.dram_tensor("b_o", [1, DIN], F32, kind="ExternalInput")
    out_d = nc.dram_tensor("out_partial", [S, DIN], F32, kind="ExternalOutput")

    # transposed views for DMA (din/hd on partitions)
    x_r = x_d.ap().rearrange("s (c p) -> c p s", p=128)      # [KC,128,S]
    wq_r = wq_d.ap().rearrange("o (c p) -> c p o", p=128)    # [KC,128,DQK]
    wk_r = wk_d.ap().rearrange("o (c p) -> c p o", p=128)
    wv_r = wv_d.ap().rearrange("o (c p) -> c p o", p=128)
    wo_r = wo_d.ap().rearrange("o (c p) -> c p o", p=128)    # [VC,128,DIN]

    with tile.TileContext(nc) as tc:
        with (
            tc.tile_pool(name="persist", bufs=1) as pp,
            tc.tile_pool(name="stage", bufs=4) as stage,
            tc.tile_pool(name="expp", bufs=4) as ep,
            tc.tile_pool(name="recp", bufs=2) as rp,
            tc.tile_pool(name="outp", bufs=4) as op_,
        ):
            # ---- persistent SBUF tensors ----
            xt = pp.tile([128, KC * S], BF16, tag="xt")          # x^T bf16
            wqk = pp.tile([128, KC * 4 * 128], BF16, tag="wqk")  # [q01 q23 k01 k23] per chunk
            wv = pp.tile([128, KC * DQK], BF16, tag="wv")        # w_v^T
            wo = pp.tile([128, VC * DIN], BF16, tag="wo")        # w_o^T
            qk = pp.tile([128, 4 * S], BF16, tag="qk")           # q01,q23,k01,k23 blocks
            vv = pp.tile([128, SB * (HPC * 65)], BF16, tag="vv")  # V'[s,4x(64+1)]
            at = pp.tile([128, 2 * S], BF16, tag="at")           # attnT pairs
            bqk = pp.tile([128, 4], F32, tag="bqk")              # per-block bias
            ones1 = pp.tile([1, 128], BF16, tag="ones")
            bv_b = pp.tile([1, DQK], BF16, tag="bvb")
            bo_b = pp.tile([1, DIN], BF16, tag="bob")

            nc.vector.memset(ones1[:], 1.0)

            # ---- load + cast inputs ----
            def load_cast(dst_ap, src_ap, cols):
                st = stage.tile([128, cols], F32, tag="stage")
                nc.sync.dma_start(st[:, :cols], src_ap)
                nc.vector.tensor_copy(dst_ap, st[:, :cols])

            # biases (small)
            nc.sync.dma_start(bqk[:, 0:1], bq_d.ap()[0:128, :])
            nc.sync.dma_start(bqk[:, 1:2], bq_d.ap()[128:256, :])
            nc.sync.dma_start(bqk[:, 2:3], bk_d.ap()[0:128, :])
            nc.sync.dma_start(bqk[:, 3:4], bk_d.ap()[128:256, :])
            stb = stage.tile([1, DQK + DIN], F32, tag="stb")
            nc.sync.dma_start(stb[:, 0:DQK], bv_d.ap())
            nc.sync.dma_start(stb[:, DQK:DQK + DIN], bo_d.ap())
            nc.vector.tensor_copy(bv_b[:], stb[:, 0:DQK])
            nc.vector.tensor_copy(bo_b[:], stb[:, DQK:DQK + DIN])

            # weights
            for c in range(KC):
                load_cast(wqk[:, c * 512:c * 512 + 256], wq_r[c], 256)
                load_cast(wqk[:, c * 512 + 256:c * 512 + 512], wk_r[c], 256)
                load_cast(wv[:, c * DQK:(c + 1) * DQK], wv_r[c], 256)
            for c in range(VC):
                for dc in range(NDC):
                    load_cast(wo[:, c * DIN + dc * ODC:c * DIN + (dc + 1) * ODC],
                              wo_r[c][:, dc * ODC:(dc + 1) * ODC], ODC)
            # x^T
            for c in range(KC):
                for sc in range(NSC):
                    load_cast(xt[:, c * S + sc * MMN:c * S + (sc + 1) * MMN],
                              x_r[c][:, sc * MMN:(sc + 1) * MMN], MMN)

            # ---- phase A: projections ----
            with (
                tc.tile_pool(name="ps_qkv", bufs=2, space="PSUM") as pq,
                tc.tile_pool(name="ps_v", bufs=2, space="PSUM") as pv,
            ):
                # Q^T, K^T  (dout blocks: q01,q23,k01,k23; wqk col layout matches)
                for blk in range(4):
                    wcol = (blk % 2) * 128 + (blk // 2) * 256  # q01,q23,k01,k23
                    for sc in range(NSC):
                        ps = pq.tile([128, MMN], F32, tag="psq")
                        for c in range(KC):
                            nc.tensor.matmul(
                                ps[:],
                                wqk[:, c * 512 + wcol:c * 512 + wcol + 128],
                                xt[:, c * S + sc * MMN:c * S + (sc + 1) * MMN],
                                start=(c == 0), stop=(c == KC - 1))
                        nc.vector.tensor_scalar_add(
                            qk[:, blk * S + sc * MMN:blk * S + (sc + 1) * MMN],
                            ps[:], bqk[:, blk:blk + 1])
                # V natural [s, 4*64] + bias, stored with stride-65 ones col
                for i in range(SB):
                    ps = pv.tile([128, DQK], F32, tag="psv")
                    for c in range(KC):
                        nc.tensor.matmul(
                            ps[:],
                            xt[:, c * S + i * 128:c * S + (i + 1) * 128],
                            wv[:, c * DQK:(c + 1) * DQK],
                            start=(c == 0), stop=False)
                    nc.tensor.matmul(ps[:], ones1[0:1, 0:128], bv_b[:],
                                     start=False, stop=True)
                    vbase = i * (HPC * 65)
                    dst = vv[:, vbase:vbase + HPC * 65]
                    dst3 = dst.rearrange("p (h d) -> p h d", h=HPC)
                    src3 = ps[:].rearrange("p (h d) -> p h d", h=HPC)
                    nc.vector.tensor_copy(dst3[:, :, 0:64], src3)
                    nc.vector.memset(dst3[:, :, 64:65], 1.0)

            # ---- phase B: attention ----
            with (
                tc.tile_pool(name="ps_s", bufs=2, space="PSUM") as psp,
                tc.tile_pool(name="ps_o", bufs=2, space="PSUM") as pop,
            ):
                for pr in range(2):          # head pair
                    qblk, kblk = pr, 2 + pr
                    for hl in range(2):      # head in pair
                        h = 2 * pr + hl
                        prow = 64 * hl
                        for sqh in range(NQ):
                            po = pop.tile([65, SQC], F32, tag="po")
                            for i in range(SB):
                                ps = psp.tile([128, SQC], F32, tag="ps")
                                for n2 in range(N2):
                                    nc.tensor.matmul(
                                        ps[:, n2 * MMN:(n2 + 1) * MMN],
                                        qk[prow:prow + 64,
                                           kblk * S + i * 128:kblk * S + (i + 1) * 128],
                                        qk[prow:prow + 64,
                                           qblk * S + sqh * SQC + n2 * MMN:
                                           qblk * S + sqh * SQC + (n2 + 1) * MMN],
                                        start=True, stop=True)
                                ex = ep.tile([128, SQC], BF16, tag="ex")
                                nc.scalar.activation(ex[:], ps[:], AF.Exp,
                                                     scale=SCALE_INV)
                                vbase = i * (HPC * 65) + h * 65
                                for n2 in range(N2):
                                    nc.tensor.matmul(
                                        po[:, n2 * MMN:(n2 + 1) * MMN],
                                        vv[:, vbase:vbase + 65],
                                        ex[:, n2 * MMN:(n2 + 1) * MMN],
                                        start=(i == 0), stop=(i == SB - 1))
                            # normalize: at[prow:prow+64, pr*S+sqh*SQC] =
                            #   po[0:64] * (1/po[64])
                            rec = rp.tile([1, SQC], F32, tag="rec")
                            nc.vector.reciprocal(rec[:], po[64:65, :])
                            r64 = rp.tile([64, SQC], F32, tag="r64")
                            if USE_GPSIMD_BCAST:
                                nc.gpsimd.partition_broadcast(r64[:], rec[:])
                            else:
                                rb = rp.tile([1, SQC], BF16, tag="recb")
                                nc.vector.tensor_copy(rb[:], rec[:])
                                pr64 = psp.tile([64, SQC], F32, tag="ps")
                                for n2 in range(N2):
                                    nc.tensor.matmul(
                                        pr64[:, n2 * MMN:(n2 + 1) * MMN],
                                        ones1[0:1, 0:64],
                                        rb[:, n2 * MMN:(n2 + 1) * MMN],
                                        start=True, stop=True)
                                nc.vector.tensor_copy(r64[:], pr64[:])
                            nc.vector.tensor_mul(
                                at[prow:prow + 64,
                                   pr * S + sqh * SQC:pr * S + (sqh + 1) * SQC],
                                po[0:64, :], r64[:])

            # ---- phase C: out projection (partial) ----
            with tc.tile_pool(name="ps_p", bufs=3, space="PSUM") as ppp:
                for sqb in range(SB):
                    for dc in range(NDC):
                        ps = ppp.tile([128, ODC], F32, tag="pp")
                        for ch in range(VC):
                            nc.tensor.matmul(
                                ps[:],
                                at[:, ch * S + sqb * 128:ch * S + (sqb + 1) * 128],
                                wo[:, ch * DIN + dc * ODC:ch * DIN + (dc + 1) * ODC],
                                start=(ch == 0), stop=False)
                        nc.tensor.matmul(ps[:], ones1[0:1, 0:128],
                                         bo_b[:, dc * ODC:(dc + 1) * ODC],
                                         start=False, stop=True)
                        ot = op_.tile([128, ODC], F32, tag="ot")
                        nc.vector.tensor_copy(ot[:], ps[:])
                        nc.sync.dma_start(
                            out_d.ap()[sqb * 128:(sqb + 1) * 128,
                                       dc * ODC:(dc + 1) * ODC],
                            ot[:])

    nc.compile()
    return nc


def shard_inputs(x, w_qkv, b_qkv, w_out, b_out, cfg: Cfg = FULL):
    """Build the 8 per-core input maps from full inputs."""
    S, DIN, DQK = cfg.S, cfg.DIN, cfg.DQK
    D = DIN
    x = np.asarray(x, dtype=np.float32)
    w_qkv = np.asarray(w_qkv, dtype=np.float32)
    b_qkv = np.asarray(b_qkv, dtype=np.float32)
    w_out = np.asarray(w_out, dtype=np.float32)
    b_out = np.asarray(b_out, dtype=np.float32)
    zeros_bo = np.zeros((1, DIN), dtype=np.float32)
    in_maps = []
    for c in range(cfg.N_CORES):
        b, hg = divmod(c, 4)
        sl = slice(hg * DQK, (hg + 1) * DQK)
        in_maps.append({
            "x": np.ascontiguousarray(x[b]),
            "w_q": np.ascontiguousarray(w_qkv[0 * D:1 * D][sl]),
            "w_k": np.ascontiguousarray(w_qkv[1 * D:2 * D][sl]),
            "w_v": np.ascontiguousarray(w_qkv[2 * D:3 * D][sl]),
            "b_q": np.ascontiguousarray(b_qkv[0 * D:1 * D][sl].reshape(DQK, 1)),
            "b_k": np.ascontiguousarray(b_qkv[1 * D:2 * D][sl].reshape(DQK, 1)),
            "b_v": np.ascontiguousarray(b_qkv[2 * D:3 * D][sl].reshape(1, DQK)),
            "w_o": np.ascontiguousarray(w_out[:, sl]),
            "b_o": (np.ascontiguousarray(b_out.reshape(1, DIN))
                    if hg == 0 else zeros_bo),
        })
    return in_maps


def gather_output(results, cfg: Cfg = FULL):
    outs = []
    for b in range(2):
        acc = results[4 * b]["out_partial"].astype(np.float32)
        for c in range(4 * b + 1, 4 * b + 4):
            acc = acc + results[c]["out_partial"]
        outs.append(acc)
    return np.stack(outs, axis=0)


_NC_CACHE = {}


def _get_nc(cfg: Cfg = FULL):
    if cfg not in _NC_CACHE:
        _NC_CACHE[cfg] = build_nc(cfg)
    return _NC_CACHE[cfg]


def kernel(x, w_qkv, b_qkv, w_out, b_out):
    cfg = FULL
    nc = _get_nc(cfg)
    in_maps = shard_inputs(x, w_qkv, b_qkv, w_out, b_out, cfg)
    res = run_bass_kernel_spmd(nc, in_maps, core_ids=list(range(cfg.N_CORES)))
    return gather_output(res.results, cfg)


if __name__ == "__main__":
    # quick self-run with random data at full size
    rng = np.random.default_rng(0)
    D = FULL.DIN
    x = rng.standard_normal((2, FULL.S, D), dtype=np.float32)
    w_qkv = (rng.standard_normal((3 * D, D), dtype=np.float32) / np.sqrt(D))
    b_qkv = rng.standard_normal(3 * D, dtype=np.float32) * 0.02
    w_out = rng.standard_normal((D, D), dtype=np.float32) / np.sqrt(D)
    b_out = rng.standard_normal(D, dtype=np.float32) * 0.02
    out = kernel(x=x, w_qkv=w_qkv, b_qkv=b_qkv, w_out=w_out, b_out=b_out)
    print("out", out.shape, out.dtype, float(np.abs(out).mean()))


# revision 11
# speedup vs baseline: 1.2460x; 1.2460x over previous
"""Trainium2 Bass kernel for 16-head MultiHeadAttention (B=2, S=2048, D=1024).

Sharding: 8 cores = 2 (batch) x 4 (head groups of 4 heads).
Each core computes qkv projection for its 4 heads, attention, and a partial
out-projection (TP over heads); host sums the 4 partials per batch element.

Self-contained: hardcodes shapes; only dependency is the in-container
concourse/bass stack at /opt/trn_rl_repo.
"""

import os
import sys
from dataclasses import dataclass

for _p in ("/opt/trn_rl_repo",):
    if _p not in sys.path:
        sys.path.insert(0, _p)

import numpy as np

import concourse.bass as bass  # noqa: E402
import concourse.bacc as bacc  # noqa: E402
import concourse.tile as tile  # noqa: E402
from concourse import mybir  # noqa: E402
from concourse.bass_utils import run_bass_kernel_spmd  # noqa: E402

F32 = mybir.dt.float32
BF16 = mybir.dt.bfloat16
AF = mybir.ActivationFunctionType

# Set False if gpsimd partition_broadcast is unavailable; falls back to a
# PE ones-matmul broadcast.
USE_GPSIMD_BCAST = True


@dataclass(frozen=True)
class Cfg:
    S: int = 2048      # sequence length
    DIN: int = 1024    # model dim
    HPC: int = 4       # heads per core
    DK: int = 64       # head dim
    N_CORES: int = 8

    @property
    def DQK(self):
        return self.HPC * self.DK  # 256: per-core Q (and K, V) output dim

    @property
    def KC(self):
        return self.DIN // 128     # contraction chunks for projections

    @property
    def SB(self):
        return self.S // 128       # 128-row blocks of the sequence

    @property
    def SQC(self):
        return min(1024, self.S)   # query-column chunk for attention

    @property
    def MMN(self):
        return min(512, self.S)    # matmul moving free-dim chunk


FULL = Cfg()


def build_nc(cfg: Cfg = FULL):
    S, DIN, HPC, DK = cfg.S, cfg.DIN, cfg.HPC, cfg.DK
    DQK, KC, SB, SQC, MMN = cfg.DQK, cfg.KC, cfg.SB, cfg.SQC, cfg.MMN
    NQ = S // SQC
    N2 = SQC // MMN          # moving chunks per SQC
    NSC = S // MMN           # s chunks for projections
    NDC = max(1, DIN // 512)  # out-proj dout chunks
    ODC = DIN // NDC
    VC = DQK // 128          # head-dim chunks for out-proj contraction (2)
    SCALE_INV = 1.0 / float(np.sqrt(DK))

    nc = bacc.Bacc("TRN2", target_bir_lowering=False, debug=False,
                   num_devices=cfg.N_CORES)

    x_d = nc.dram_tensor("x", [S, DIN], F32, kind="ExternalInput")
    wq_d = nc.dram_tensor("w_q", [DQK, DIN], F32, kind="ExternalInput")
    wk_d = nc.dram_tensor("w_k", [DQK, DIN], F32, kind="ExternalInput")
    wv_d = nc.dram_tensor("w_v", [DQK, DIN], F32, kind="ExternalInput")
    bq_d = nc.dram_tensor("b_q", [DQK, 1], F32, kind="ExternalInput")
    bk_d = nc.dram_tensor("b_k", [DQK, 1], F32, kind="ExternalInput")
    bv_d = nc.dram_tensor("b_v", [1, DQK], F32, kind="ExternalInput")
    wo_d = nc.dram_tensor("w_o", [DIN, DQK], F32, kind="ExternalInput")
    bo_d = nc.dram_tensor("b_o", [1, DIN], F32, kind="ExternalInput")
    out_d = nc.dram_tensor("out_partial", [S, DIN], F32, kind="ExternalOutput")

    with tile.TileContext(nc) as tc:
        with (
            tc.tile_pool(name="persist", bufs=1) as pp,
            tc.tile_pool(name="stage", bufs=6) as stage,
            tc.tile_pool(name="natp", bufs=20) as natp,
            tc.tile_pool(name="expp", bufs=6) as ep,
            tc.tile_pool(name="recp", bufs=2) as rp,
            tc.tile_pool(name="outp", bufs=4) as op_,
        ):
            # ---- persistent SBUF tensors ----
            # xt col layout: c*S + s  (chunk-major; contiguous s for matmul rhs)
            xt = pp.tile([128, SB * DIN], BF16, tag="xt")        # x^T bf16
            # wqk col layout: blk*DIN + c*128 + dout_w; blk in q01,q23,k01,k23
            wqk = pp.tile([128, 4 * DIN], BF16, tag="wqk")
            wv = pp.tile([128, VC * DIN], BF16, tag="wv")        # c*DQK + dout
            wo = pp.tile([128, VC * DIN], BF16, tag="wo")        # ch*DIN + dout
            qk = pp.tile([128, 4 * S], BF16, tag="qk")           # q01,q23,k01,k23 blocks
            vv = pp.tile([128, SB * (HPC * 65)], BF16, tag="vv")  # V'[s,4x(64+1)]
            at = pp.tile([128, 2 * S], BF16, tag="at")           # attnT pairs
            bqk = pp.tile([128, 4], F32, tag="bqk")              # per-block bias
            ones1 = pp.tile([1, 128], BF16, tag="ones")
            bv_b = pp.tile([1, DQK], BF16, tag="bvb")
            bo_b = pp.tile([1, DIN], BF16, tag="bob")

            xtc = xt[:].rearrange("p (c s) -> p c s", c=KC)      # [128,KC,S]
            wvc = wv[:].rearrange("p (c d) -> p c d", c=KC)      # [128,KC,DQK]
            wov = wo[:].rearrange("p (ch d) -> p ch d", ch=VC)   # [128,VC,DIN]

            nc.vector.memset(ones1[:], 1.0)

            # ---- contiguous loads + cast to bf16 + xbar transpose ----
            # biases (small)
            nc.sync.dma_start(bqk[:, 0:1], bq_d.ap()[0:128, :])
            nc.sync.dma_start(bqk[:, 1:2], bq_d.ap()[128:256, :])
            nc.sync.dma_start(bqk[:, 2:3], bk_d.ap()[0:128, :])
            nc.sync.dma_start(bqk[:, 3:4], bk_d.ap()[128:256, :])
            stb = stage.tile([1, DQK + DIN], F32, tag="stb", bufs=1)
            nc.sync.dma_start(stb[:, 0:DQK], bv_d.ap())
            nc.sync.dma_start(stb[:, DQK:DQK + DIN], bo_d.ap())
            nc.vector.tensor_copy(bv_b[:], stb[:, 0:DQK])
            nc.vector.tensor_copy(bo_b[:], stb[:, DQK:DQK + DIN])

            cast_flip = [0]

            def load_cast(src2d, rows, cols):
                st = stage.tile([128, cols], F32, tag="stage")
                nc.sync.dma_start(st[:rows, :], src2d)
                nb = natp.tile([128, cols], BF16, tag="nat")
                # alternate cast engine: ACT is idle during the load phase
                if cast_flip[0] % 2 == 0:
                    nc.vector.tensor_copy(nb[:rows, :], st[:rows, :])
                else:
                    nc.scalar.copy(nb[:rows, :], st[:rows, :])
                cast_flip[0] += 1
                return nb

            # group 1: qkv weights + first 4 x blocks, then their transposes
            nb_wq = [load_cast(wq_d.ap()[b * 128:(b + 1) * 128, :], 128, DIN)
                     for b in range(2)]
            nb_wk = [load_cast(wk_d.ap()[b * 128:(b + 1) * 128, :], 128, DIN)
                     for b in range(2)]
            nb_wv = [load_cast(wv_d.ap()[b * 128:(b + 1) * 128, :], 128, DIN)
                     for b in range(2)]
            nb_x = {}
            for i in range(min(4, SB)):
                nb_x[i] = load_cast(x_d.ap()[i * 128:(i + 1) * 128, :], 128, DIN)
            for b in range(2):
                nc.sync.dma_start_transpose(
                    wqk[:, b * DIN:(b + 1) * DIN]
                    .rearrange("p (c s) -> p c s", c=KC), nb_wq[b][:])
                nc.sync.dma_start_transpose(
                    wqk[:, (2 + b) * DIN:(3 + b) * DIN]
                    .rearrange("p (c s) -> p c s", c=KC), nb_wk[b][:])
                nc.sync.dma_start_transpose(
                    wvc[:, :, b * 128:(b + 1) * 128], nb_wv[b][:])
            for i in range(min(4, SB)):
                nc.sync.dma_start_transpose(
                    xtc[:, :, i * 128:(i + 1) * 128], nb_x[i][:])

            # group 2: remaining x + w_o loads, then transposes
            for i in range(4, SB):
                nb_x[i] = load_cast(x_d.ap()[i * 128:(i + 1) * 128, :], 128, DIN)
            nb_wo = [load_cast(wo_d.ap()[b * 128:(b + 1) * 128, :], 128, DQK)
                     for b in range(DIN // 128)]
            for i in range(4, SB):
                nc.sync.dma_start_transpose(
                    xtc[:, :, i * 128:(i + 1) * 128], nb_x[i][:])
            for b in range(DIN // 128):
                nc.sync.dma_start_transpose(
                    wov[:, :, b * 128:(b + 1) * 128], nb_wo[b][:])

            # ---- phase A: projections ----
            with (
                tc.tile_pool(name="ps_qkv", bufs=2, space="PSUM") as pq,
                tc.tile_pool(name="ps_v", bufs=2, space="PSUM") as pv,
            ):
                # per s-chunk: Q^T/K^T blocks then V blocks (early start)
                BPM = MMN // 128
                for sc in range(NSC):
                    for blk in range(4):
                        ps = pq.tile([128, MMN], F32, tag="psq")
                        for c in range(KC):
                            nc.tensor.matmul(
                                ps[:],
                                wqk[:, blk * DIN + c * 128:blk * DIN + (c + 1) * 128],
                                xt[:, c * S + sc * MMN:c * S + (sc + 1) * MMN],
                                start=(c == 0), stop=(c == KC - 1))
                        nc.vector.tensor_scalar_add(
                            qk[:, blk * S + sc * MMN:blk * S + (sc + 1) * MMN],
                            ps[:], bqk[:, blk:blk + 1])
                    # V natural [s, 4*64] + bias, stride-65 ones col
                    for i in range(sc * BPM, (sc + 1) * BPM):
                        ps = pv.tile([128, DQK], F32, tag="psv")
                        for c in range(KC):
                            nc.tensor.matmul(
                                ps[:],
                                xt[:, c * S + i * 128:c * S + (i + 1) * 128],
                                wv[:, c * DQK:(c + 1) * DQK],
                                start=(c == 0), stop=False)
                        nc.tensor.matmul(ps[:], ones1[0:1, 0:128], bv_b[:],
                                         start=False, stop=True)
                        vbase = i * (HPC * 65)
                        dst = vv[:, vbase:vbase + HPC * 65]
                        dst3 = dst.rearrange("p (h d) -> p h d", h=HPC)
                        src3 = ps[:].rearrange("p (h d) -> p h d", h=HPC)
                        nc.vector.tensor_copy(dst3[:, :, 0:64], src3)
                        nc.vector.memset(dst3[:, :, 64:65], 1.0)

            # ---- phase B: attention (two heads of a pair interleaved) ----
            with (
                tc.tile_pool(name="ps_s", bufs=2, space="PSUM") as psp,
                tc.tile_pool(name="ps_o", bufs=2, space="PSUM") as pop,
            ):
                for sqh in range(NQ):
                    for pr in range(2):          # head pair
                        qblk, kblk = pr, 2 + pr
                        po = [pop.tile([65, SQC], F32, tag="po",
                                       name=f"po{hl}") for hl in range(2)]
                        for i in range(SB):
                            ps = [psp.tile([128, SQC], F32, tag="ps",
                                           name=f"ps{hl}") for hl in range(2)]
                            for n2 in range(N2):
                                for hl in range(2):
                                    prow = 64 * hl
                                    nc.tensor.matmul(
                                        ps[hl][:, n2 * MMN:(n2 + 1) * MMN],
                                        qk[prow:prow + 64,
                                           kblk * S + i * 128:kblk * S + (i + 1) * 128],
                                        qk[prow:prow + 64,
                                           qblk * S + sqh * SQC + n2 * MMN:
                                           qblk * S + sqh * SQC + (n2 + 1) * MMN],
                                        start=True, stop=True)
                            ex = [None, None]
                            for hl in range(2):
                                ex[hl] = ep.tile([128, SQC], BF16, tag="ex",
                                                 name=f"ex{hl}")
                                nc.scalar.activation(ex[hl][:], ps[hl][:],
                                                     AF.Exp, scale=SCALE_INV)
                            for hl in range(2):
                                h = 2 * pr + hl
                                vbase = i * (HPC * 65) + h * 65
                                for n2 in range(N2):
                                    nc.tensor.matmul(
                                        po[hl][:, n2 * MMN:(n2 + 1) * MMN],
                                        vv[:, vbase:vbase + 65],
                                        ex[hl][:, n2 * MMN:(n2 + 1) * MMN],
                                        start=(i == 0), stop=(i == SB - 1))
                        # normalize: at[prow:+64, pr*S+sqh*SQC] = po[0:64]/po[64]
                        for hl in range(2):
                            prow = 64 * hl
                            rec = rp.tile([1, SQC], F32, tag="rec")
                            nc.vector.reciprocal(rec[:], po[hl][64:65, :])
                            r64 = rp.tile([64, SQC], F32, tag="r64")
                            if USE_GPSIMD_BCAST:
                                nc.gpsimd.partition_broadcast(r64[:], rec[:])
                            else:
                                rb = rp.tile([1, SQC], BF16, tag="recb")
                                nc.vector.tensor_copy(rb[:], rec[:])
                                pr64 = psp.tile([64, SQC], F32, tag="ps")
                                for n2 in range(N2):
                                    nc.tensor.matmul(
                                        pr64[:, n2 * MMN:(n2 + 1) * MMN],
                                        ones1[0:1, 0:64],
                                        rb[:, n2 * MMN:(n2 + 1) * MMN],
                                        start=True, stop=True)
                                nc.vector.tensor_copy(r64[:], pr64[:])
                            nc.vector.tensor_mul(
                                at[prow:prow + 64,
                                   pr * S + sqh * SQC:pr * S + (sqh + 1) * SQC],
                                po[hl][0:64, :], r64[:])

            # ---- phase C: out projection (partial) ----
            with tc.tile_pool(name="ps_p", bufs=3, space="PSUM") as ppp:
                for sqb in range(SB):
                    for dc in range(NDC):
                        ps = ppp.tile([128, ODC], F32, tag="pp")
                        for ch in range(VC):
                            nc.tensor.matmul(
                                ps[:],
                                at[:, ch * S + sqb * 128:ch * S + (sqb + 1) * 128],
                                wov[:, ch, dc * ODC:(dc + 1) * ODC],
                                start=(ch == 0), stop=False)
                        nc.tensor.matmul(ps[:], ones1[0:1, 0:128],
                                         bo_b[:, dc * ODC:(dc + 1) * ODC],
                                         start=False, stop=True)
                        ot = op_.tile([128, ODC], F32, tag="ot")
                        nc.vector.tensor_copy(ot[:], ps[:])
                        nc.sync.dma_start(
                            out_d.ap()[sqb * 128:(sqb + 1) * 128,
                                       dc * ODC:(dc + 1) * ODC],
                            ot[:])

    nc.compile()
    return nc


def shard_inputs(x, w_qkv, b_qkv, w_out, b_out, cfg: Cfg = FULL):
    """Build the 8 per-core input maps from full inputs."""
    S, DIN, DQK = cfg.S, cfg.DIN, cfg.DQK
    D = DIN
    x = np.asarray(x, dtype=np.float32)
    w_qkv = np.asarray(w_qkv, dtype=np.float32)
    b_qkv = np.asarray(b_qkv, dtype=np.float32)
    w_out = np.asarray(w_out, dtype=np.float32)
    b_out = np.asarray(b_out, dtype=np.float32)
    zeros_bo = np.zeros((1, DIN), dtype=np.float32)
    in_maps = []
    for c in range(cfg.N_CORES):
        b, hg = divmod(c, 4)
        sl = slice(hg * DQK, (hg + 1) * DQK)
        in_maps.append({
            "x": np.ascontiguousarray(x[b]),
            "w_q": np.ascontiguousarray(w_qkv[0 * D:1 * D][sl]),
            "w_k": np.ascontiguousarray(w_qkv[1 * D:2 * D][sl]),
            "w_v": np.ascontiguousarray(w_qkv[2 * D:3 * D][sl]),
            "b_q": np.ascontiguousarray(b_qkv[0 * D:1 * D][sl].reshape(DQK, 1)),
            "b_k": np.ascontiguousarray(b_qkv[1 * D:2 * D][sl].reshape(DQK, 1)),
            "b_v": np.ascontiguousarray(b_qkv[2 * D:3 * D][sl].reshape(1, DQK)),
            "w_o": np.ascontiguousarray(w_out[:, sl]),
            "b_o": (np.ascontiguousarray(b_out.reshape(1, DIN))
                    if hg == 0 else zeros_bo),
        })
    return in_maps


def gather_output(results, cfg: Cfg = FULL):
    outs = []
    for b in range(2):
        acc = results[4 * b]["out_partial"].astype(np.float32)
        for c in range(4 * b + 1, 4 * b + 4):
            acc = acc + results[c]["out_partial"]
        outs.append(acc)
    return np.stack(outs, axis=0)


_NC_CACHE = {}


def _get_nc(cfg: Cfg = FULL):
    if cfg not in _NC_CACHE:
        _NC_CACHE[cfg] = build_nc(cfg)
    return _NC_CACHE[cfg]


def kernel(x, w_qkv, b_qkv, w_out, b_out):
    cfg = FULL
    nc = _get_nc(cfg)
    in_maps = shard_inputs(x, w_qkv, b_qkv, w_out, b_out, cfg)
    res = run_bass_kernel_spmd(nc, in_maps, core_ids=list(range(cfg.N_CORES)))
    return gather_output(res.results, cfg)


if __name__ == "__main__":
    # quick self-run with random data at full size
    rng = np.random.default_rng(0)
    D = FULL.DIN
    x = rng.standard_normal((2, FULL.S, D), dtype=np.float32)
    w_qkv = (rng.standard_normal((3 * D, D), dtype=np.float32) / np.sqrt(D))
    b_qkv = rng.standard_normal(3 * D, dtype=np.float32) * 0.02
    w_out = rng.standard_normal((D, D), dtype=np.float32) / np.sqrt(D)
    b_out = rng.standard_normal(D, dtype=np.float32) * 0.02
    out = kernel(x=x, w_qkv=w_qkv, b_qkv=b_qkv, w_out=w_out, b_out=b_out)
    print("out", out.shape, out.dtype, float(np.abs(out).mean()))


# revision 14
# speedup vs baseline: 258.7912x; 207.7052x over previous
"""Trainium2 Bass kernel for 16-head MultiHeadAttention (B=2, S=2048, D=1024).

Sharding: 8 cores = 2 (batch) x 4 (head groups of 4 heads).
Each core computes qkv projection for its 4 heads, attention, and a partial
out-projection (TP over heads); host sums the 4 partials per batch element.

Self-contained: hardcodes shapes; only dependency is the in-container
concourse/bass stack at /opt/trn_rl_repo.
"""

import os
import sys
from dataclasses import dataclass

for _p in ("/opt/trn_rl_repo",):
    if _p not in sys.path:
        sys.path.insert(0, _p)

import numpy as np

import concourse.bass as bass  # noqa: E402
import concourse.bacc as bacc  # noqa: E402
import concourse.tile as tile  # noqa: E402
from concourse import mybir  # noqa: E402
from concourse.bass_utils import run_bass_kernel_spmd  # noqa: E402

F32 = mybir.dt.float32
BF16 = mybir.dt.bfloat16
AF = mybir.ActivationFunctionType

# Set False if gpsimd partition_broadcast is unavailable; falls back to a
# PE ones-matmul broadcast.
USE_GPSIMD_BCAST = True


@dataclass(frozen=True)
class Cfg:
    S: int = 2048      # sequence length
    DIN: int = 1024    # model dim
    HPC: int = 4       # heads per core
    DK: int = 64       # head dim
    N_CORES: int = 8

    @property
    def DQK(self):
        return self.HPC * self.DK  # 256: per-core Q (and K, V) output dim

    @property
    def KC(self):
        return self.DIN // 128     # contraction chunks for projections

    @property
    def SB(self):
        return self.S // 128       # 128-row blocks of the sequence

    @property
    def SQC(self):
        return min(1024, self.S)   # query-column chunk for attention

    @property
    def MMN(self):
        return min(512, self.S)    # matmul moving free-dim chunk


FULL = Cfg()


def build_nc(cfg: Cfg = FULL):
    S, DIN, HPC, DK = cfg.S, cfg.DIN, cfg.HPC, cfg.DK
    DQK, KC, SB, SQC, MMN = cfg.DQK, cfg.KC, cfg.SB, cfg.SQC, cfg.MMN
    NQ = S // SQC
    N2 = SQC // MMN          # moving chunks per SQC
    NSC = S // MMN           # s chunks for projections
    NDC = max(1, DIN // 512)  # out-proj dout chunks
    ODC = DIN // NDC
    VC = DQK // 128          # head-dim chunks for out-proj contraction (2)
    SCALE_INV = 1.0 / float(np.sqrt(DK))

    nc = bacc.Bacc("TRN2", target_bir_lowering=False, debug=False,
                   num_devices=cfg.N_CORES)

    x_d = nc.dram_tensor("x", [S, DIN], F32, kind="ExternalInput")
    wq_d = nc.dram_tensor("w_q", [DQK, DIN], F32, kind="ExternalInput")
    wk_d = nc.dram_tensor("w_k", [DQK, DIN], F32, kind="ExternalInput")
    wv_d = nc.dram_tensor("w_v", [DQK, DIN], F32, kind="ExternalInput")
    bq_d = nc.dram_tensor("b_q", [DQK, 1], F32, kind="ExternalInput")
    bk_d = nc.dram_tensor("b_k", [DQK, 1], F32, kind="ExternalInput")
    bv_d = nc.dram_tensor("b_v", [1, DQK], F32, kind="ExternalInput")
    wo_d = nc.dram_tensor("w_o", [DIN, DQK], F32, kind="ExternalInput")
    bo_d = nc.dram_tensor("b_o", [1, DIN], F32, kind="ExternalInput")
    out_d = nc.dram_tensor("out_partial", [S, DIN], F32, kind="ExternalOutput")

    with tile.TileContext(nc) as tc:
        with (
            tc.tile_pool(name="persist", bufs=1) as pp,
            tc.tile_pool(name="stage", bufs=6) as stage,
            tc.tile_pool(name="natp", bufs=18) as natp,
            tc.tile_pool(name="expp", bufs=6) as ep,
            tc.tile_pool(name="recp", bufs=2) as rp,
            tc.tile_pool(name="outp", bufs=4) as op_,
        ):
            # ---- persistent SBUF tensors ----
            # xt col layout: c*S + s  (chunk-major; contiguous s for matmul rhs)
            xt = pp.tile([128, SB * DIN], BF16, tag="xt")        # x^T bf16
            # wqk col layout: blk*DIN + c*128 + dout_w; blk in q01,q23,k01,k23
            wqk = pp.tile([128, 4 * DIN], BF16, tag="wqk")
            wv = pp.tile([128, VC * DIN], BF16, tag="wv")        # c*DQK + dout
            wo = pp.tile([128, VC * DIN], BF16, tag="wo")        # ch*DIN + dout
            qk = pp.tile([128, 4 * S], BF16, tag="qk")           # q01,q23,k01,k23 blocks
            vv = pp.tile([128, SB * (HPC * 65)], BF16, tag="vv")  # V'[s,4x(64+1)]
            at = pp.tile([128, 2 * S], BF16, tag="at")           # attnT pairs
            bqk = pp.tile([128, 4], F32, tag="bqk")              # per-block bias
            ones1 = pp.tile([1, 128], BF16, tag="ones")
            bv_b = pp.tile([1, DQK], BF16, tag="bvb")
            bo_b = pp.tile([1, DIN], BF16, tag="bob")

            xtc = xt[:].rearrange("p (c s) -> p c s", c=KC)      # [128,KC,S]
            wvc = wv[:].rearrange("p (c d) -> p c d", c=KC)      # [128,KC,DQK]
            wov = wo[:].rearrange("p (ch d) -> p ch d", ch=VC)   # [128,VC,DIN]

            nc.vector.memset(ones1[:], 1.0)

            # ---- contiguous loads + cast to bf16 + xbar transpose ----
            # biases (small)
            nc.sync.dma_start(bqk[:, 0:1], bq_d.ap()[0:128, :])
            nc.sync.dma_start(bqk[:, 1:2], bq_d.ap()[128:256, :])
            nc.sync.dma_start(bqk[:, 2:3], bk_d.ap()[0:128, :])
            nc.sync.dma_start(bqk[:, 3:4], bk_d.ap()[128:256, :])
            stb = stage.tile([1, DQK + DIN], F32, tag="stb", bufs=1)
            nc.sync.dma_start(stb[:, 0:DQK], bv_d.ap())
            nc.sync.dma_start(stb[:, DQK:DQK + DIN], bo_d.ap())
            nc.vector.tensor_copy(bv_b[:], stb[:, 0:DQK])
            nc.vector.tensor_copy(bo_b[:], stb[:, DQK:DQK + DIN])
            bo128 = pp.tile([128, DIN], F32, tag="bo128")
            assert USE_GPSIMD_BCAST, "bias bcast fallback not implemented"
            nc.gpsimd.partition_broadcast(bo128[:], stb[:, DQK:DQK + DIN])

            cast_flip = [0]

            def load_cast(src2d, rows, cols):
                st = stage.tile([128, cols], F32, tag="stage")
                nc.sync.dma_start(st[:rows, :], src2d)
                nb = natp.tile([128, cols], BF16, tag="nat")
                # alternate cast engine: ACT is idle during the load phase
                if cast_flip[0] % 2 == 0:
                    nc.vector.tensor_copy(nb[:rows, :], st[:rows, :])
                else:
                    nc.scalar.copy(nb[:rows, :], st[:rows, :])
                cast_flip[0] += 1
                return nb

            # group 1: qkv weights + first 4 x blocks, then their transposes
            nb_wq = [load_cast(wq_d.ap()[b * 128:(b + 1) * 128, :], 128, DIN)
                     for b in range(2)]
            nb_wk = [load_cast(wk_d.ap()[b * 128:(b + 1) * 128, :], 128, DIN)
                     for b in range(2)]
            nb_wv = [load_cast(wv_d.ap()[b * 128:(b + 1) * 128, :], 128, DIN)
                     for b in range(2)]
            nb_x = {}
            for i in range(min(4, SB)):
                nb_x[i] = load_cast(x_d.ap()[i * 128:(i + 1) * 128, :], 128, DIN)
            for b in range(2):
                nc.sync.dma_start_transpose(
                    wqk[:, b * DIN:(b + 1) * DIN]
                    .rearrange("p (c s) -> p c s", c=KC), nb_wq[b][:])
                nc.sync.dma_start_transpose(
                    wqk[:, (2 + b) * DIN:(3 + b) * DIN]
                    .rearrange("p (c s) -> p c s", c=KC), nb_wk[b][:])
                nc.sync.dma_start_transpose(
                    wvc[:, :, b * 128:(b + 1) * 128], nb_wv[b][:])
            for i in range(min(4, SB)):
                nc.sync.dma_start_transpose(
                    xtc[:, :, i * 128:(i + 1) * 128], nb_x[i][:])

            # group 2: remaining x + w_o loads, then transposes
            for i in range(4, SB):
                nb_x[i] = load_cast(x_d.ap()[i * 128:(i + 1) * 128, :], 128, DIN)
            nb_wo = [load_cast(wo_d.ap()[b * 128:(b + 1) * 128, :], 128, DQK)
                     for b in range(DIN // 128)]
            for i in range(4, SB):
                nc.sync.dma_start_transpose(
                    xtc[:, :, i * 128:(i + 1) * 128], nb_x[i][:])
            for b in range(DIN // 128):
                nc.sync.dma_start_transpose(
                    wov[:, :, b * 128:(b + 1) * 128], nb_wo[b][:])

            # ---- phase A: projections ----
            with (
                tc.tile_pool(name="ps_qkv", bufs=2, space="PSUM") as pq,
                tc.tile_pool(name="ps_v", bufs=2, space="PSUM") as pv,
            ):
                # per s-chunk: Q^T/K^T blocks then V blocks (early start)
                BPM = MMN // 128
                for sc in range(NSC):
                    for blk in range(4):
                        ps = pq.tile([128, MMN], F32, tag="psq")
                        for c in range(KC):
                            nc.tensor.matmul(
                                ps[:],
                                wqk[:, blk * DIN + c * 128:blk * DIN + (c + 1) * 128],
                                xt[:, c * S + sc * MMN:c * S + (sc + 1) * MMN],
                                start=(c == 0), stop=(c == KC - 1))
                        nc.vector.tensor_scalar_add(
                            qk[:, blk * S + sc * MMN:blk * S + (sc + 1) * MMN],
                            ps[:], bqk[:, blk:blk + 1])
                    # V natural [s, 4*64] + bias, stride-65 ones col
                    for i in range(sc * BPM, (sc + 1) * BPM):
                        ps = pv.tile([128, DQK], F32, tag="psv")
                        for c in range(KC):
                            nc.tensor.matmul(
                                ps[:],
                                xt[:, c * S + i * 128:c * S + (i + 1) * 128],
                                wv[:, c * DQK:(c + 1) * DQK],
                                start=(c == 0), stop=False)
                        nc.tensor.matmul(ps[:], ones1[0:1, 0:128], bv_b[:],
                                         start=False, stop=True)
                        vbase = i * (HPC * 65)
                        dst = vv[:, vbase:vbase + HPC * 65]
                        dst3 = dst.rearrange("p (h d) -> p h d", h=HPC)
                        src3 = ps[:].rearrange("p (h d) -> p h d", h=HPC)
                        nc.vector.tensor_copy(dst3[:, :, 0:64], src3)
                        nc.vector.memset(dst3[:, :, 64:65], 1.0)

            # ---- phase B: attention (two heads of a pair interleaved) ----
            with (
                tc.tile_pool(name="ps_s", bufs=2, space="PSUM") as psp,
                tc.tile_pool(name="ps_o", bufs=2, space="PSUM") as pop,
            ):
                for sqh in range(NQ):
                    for pr in range(2):          # head pair
                        qblk, kblk = pr, 2 + pr
                        po = [pop.tile([65, SQC], F32, tag="po",
                                       name=f"po{hl}") for hl in range(2)]
                        for i in range(SB):
                            ps = [psp.tile([128, SQC], F32, tag="ps",
                                           name=f"ps{hl}") for hl in range(2)]
                            for n2 in range(N2):
                                for hl in range(2):
                                    prow = 64 * hl
                                    nc.tensor.matmul(
                                        ps[hl][:, n2 * MMN:(n2 + 1) * MMN],
                                        qk[prow:prow + 64,
                                           kblk * S + i * 128:kblk * S + (i + 1) * 128],
                                        qk[prow:prow + 64,
                                           qblk * S + sqh * SQC + n2 * MMN:
                                           qblk * S + sqh * SQC + (n2 + 1) * MMN],
                                        start=True, stop=True)
                            ex = [None, None]
                            for hl in range(2):
                                ex[hl] = ep.tile([128, SQC], BF16, tag="ex",
                                                 name=f"ex{hl}")
                                nc.scalar.activation(ex[hl][:], ps[hl][:],
                                                     AF.Exp, scale=SCALE_INV)
                            for hl in range(2):
                                h = 2 * pr + hl
                                vbase = i * (HPC * 65) + h * 65
                                for n2 in range(N2):
                                    nc.tensor.matmul(
                                        po[hl][:, n2 * MMN:(n2 + 1) * MMN],
                                        vv[:, vbase:vbase + 65],
                                        ex[hl][:, n2 * MMN:(n2 + 1) * MMN],
                                        start=(i == 0), stop=(i == SB - 1))
                        # evacuate raw PV output (frees PSUM fast), then
                        # normalize from SBUF: at = atu[0:64] / atu[64]
                        for hl in range(2):
                            prow = 64 * hl
                            atu = rp.tile([65, SQC], F32, tag="atu",
                                          name=f"atu{hl}")
                            nc.vector.tensor_copy(atu[:], po[hl][:])
                            rec = rp.tile([1, SQC], F32, tag="rec")
                            nc.vector.reciprocal(rec[:], atu[64:65, :])
                            r64 = rp.tile([64, SQC], F32, tag="r64")
                            if USE_GPSIMD_BCAST:
                                nc.gpsimd.partition_broadcast(r64[:], rec[:])
                            else:
                                rb = rp.tile([1, SQC], BF16, tag="recb")
                                nc.vector.tensor_copy(rb[:], rec[:])
                                pr64 = psp.tile([64, SQC], F32, tag="ps")
                                for n2 in range(N2):
                                    nc.tensor.matmul(
                                        pr64[:, n2 * MMN:(n2 + 1) * MMN],
                                        ones1[0:1, 0:64],
                                        rb[:, n2 * MMN:(n2 + 1) * MMN],
                                        start=True, stop=True)
                                nc.vector.tensor_copy(r64[:], pr64[:])
                            nc.vector.tensor_mul(
                                at[prow:prow + 64,
                                   pr * S + sqh * SQC:pr * S + (sqh + 1) * SQC],
                                atu[0:64, :], r64[:])

            # ---- phase C: out projection (partial) ----
            with tc.tile_pool(name="ps_p", bufs=3, space="PSUM") as ppp:
                for sqb in range(SB):
                    for dc in range(NDC):
                        ps = ppp.tile([128, ODC], F32, tag="pp")
                        for ch in range(VC):
                            nc.tensor.matmul(
                                ps[:],
                                at[:, ch * S + sqb * 128:ch * S + (sqb + 1) * 128],
                                wov[:, ch, dc * ODC:(dc + 1) * ODC],
                                start=(ch == 0), stop=(ch == VC - 1))
                        ot = op_.tile([128, ODC], F32, tag="ot")
                        nc.vector.tensor_add(ot[:], ps[:],
                                             bo128[:, dc * ODC:(dc + 1) * ODC])
                        nc.sync.dma_start(
                            out_d.ap()[sqb * 128:(sqb + 1) * 128,
                                       dc * ODC:(dc + 1) * ODC],
                            ot[:])

    nc.compile()
    return nc


def shard_inputs(x, w_qkv, b_qkv, w_out, b_out, cfg: Cfg = FULL):
    """Build the 8 per-core input maps from full inputs."""
    S, DIN, DQK = cfg.S, cfg.DIN, cfg.DQK
    D = DIN
    x = np.asarray(x, dtype=np.float32)
    w_qkv = np.asarray(w_qkv, dtype=np.float32)
    b_qkv = np.asarray(b_qkv, dtype=np.float32)
    w_out = np.asarray(w_out, dtype=np.float32)
    b_out = np.asarray(b_out, dtype=np.float32)
    zeros_bo = np.zeros((1, DIN), dtype=np.float32)
    in_maps = []
    for c in range(cfg.N_CORES):
        b, hg = divmod(c, 4)
        sl = slice(hg * DQK, (hg + 1) * DQK)
        in_maps.append({
            "x": np.ascontiguousarray(x[b]),
            "w_q": np.ascontiguousarray(w_qkv[0 * D:1 * D][sl]),
            "w_k": np.ascontiguousarray(w_qkv[1 * D:2 * D][sl]),
            "w_v": np.ascontiguousarray(w_qkv[2 * D:3 * D][sl]),
            "b_q": np.ascontiguousarray(b_qkv[0 * D:1 * D][sl].reshape(DQK, 1)),
            "b_k": np.ascontiguousarray(b_qkv[1 * D:2 * D][sl].reshape(DQK, 1)),
            "b_v": np.ascontiguousarray(b_qkv[2 * D:3 * D][sl].reshape(1, DQK)),
            "w_o": np.ascontiguousarray(w_out[:, sl]),
            "b_o": (np.ascontiguousarray(b_out.reshape(1, DIN))
                    if hg == 0 else zeros_bo),
        })
    return in_maps


def gather_output(results, cfg: Cfg = FULL):
    outs = []
    for b in range(2):
        acc = results[4 * b]["out_partial"].astype(np.float32)
        for c in range(4 * b + 1, 4 * b + 4):
            acc = acc + results[c]["out_partial"]
        outs.append(acc)
    return np.stack(outs, axis=0)


_NC_CACHE = {}


def _get_nc(cfg: Cfg = FULL):
    if cfg not in _NC_CACHE:
        _NC_CACHE[cfg] = build_nc(cfg)
    return _NC_CACHE[cfg]


def kernel(x, w_qkv, b_qkv, w_out, b_out):
    cfg = FULL
    nc = _get_nc(cfg)
    in_maps = shard_inputs(x, w_qkv, b_qkv, w_out, b_out, cfg)
    res = run_bass_kernel_spmd(nc, in_maps, core_ids=list(range(cfg.N_CORES)))
    return gather_output(res.results, cfg)


if __name__ == "__main__":
    # quick self-run with random data at full size
    rng = np.random.default_rng(0)
    D = FULL.DIN
    x = rng.standard_normal((2, FULL.S, D), dtype=np.float32)
    w_qkv = (rng.standard_normal((3 * D, D), dtype=np.float32) / np.sqrt(D))
    b_qkv = rng.standard_normal(3 * D, dtype=np.float32) * 0.02
    w_out = rng.standard_normal((D, D), dtype=np.float32) / np.sqrt(D)
    b_out = rng.standard_normal(D, dtype=np.float32) * 0.02
    out = kernel(x=x, w_qkv=w_qkv, b_qkv=b_qkv, w_out=w_out, b_out=b_out)
    print("out", out.shape, out.dtype, float(np.abs(out).mean()))
